# revision 15
# baseline (speedup 1.0000x reference)
"""DifferentiableTokenSelection Trainium2 kernel.

Math (reference):
    x: [b=2, t=64, n=1024, e=512] -> x_flat [b, m=65536, e]
    scores  = x_flat @ W.T + bias            [b, m, k=256]
    weights = softmax(scores / tau, axis=m)  (tau = 1.0)
    out     = einsum('bmk,bme->bke', weights, x_flat)   [b, 256, 512]

Key simplifications (exact, not approximations):
  * softmax over the m axis is invariant to any per-(b,k) constant shift,
    so the bias term cancels entirely -> ignore b_bias.
  * scores are ~N(0,1) (W ~ randn/sqrt(e), x ~ randn), max |s| ~ 6, so
    exp() without max-subtraction is safe in fp32. This makes the kernel a
    single streaming pass: U[k,e] = sum_m exp(s[m,k]) x[m,e] accumulated in
    PSUM, denom[k] = sum_m exp(s[m,k]), out = U / denom.

Matmuls run in FP32R (fp32 with mantissa RNE-rounded to 11 bits) — the
TRN2 full-rate fp32 streaming mode (1 cycle/row at free-dim >= 256 vs 4
for plain fp32). The BIR verifier requires every f32r matmul input to be
produced rounded, so the host pre-rounds x/W bits (verified bit-exact
against libwalrus fp32_to_fp32r); DMA of pre-rounded bits is accepted.

Sharding: batch x token-axis. core i handles batch i//4, m-rows
[16384*(i%4), 16384*(i%4+1)). Each core emits partial U and denom; the
host sums the 4 partials per batch and divides (gather/unshard step).

Per-core pipeline over 128-row subtiles:
  DMA x block -> PE transpose (f32r, via identity) -> PSUM -> DVE copy
  to SBUF -> mm1 (f32r): scores[m,k] += xT_chunk^T @ WT_chunk ->
  ACT exp -> mm2 (f32r): U[k,e] += wexp_chunk^T @ x, den[k] += wexp^T @ 1.
All constants ride one DMA and the xT copies stay on one engine to keep
per-matmul semaphore-wait fan-in within the ISA limit.
"""

import numpy as np

import concourse.bacc as bacc
import concourse.bass as bass
import concourse.tile as tile
from concourse import mybir
from concourse.bass_utils import run_bass_kernel_spmd

B, T, NTOK, E, K = 2, 64, 1024, 512, 256
M = T * NTOK                 # 65536 tokens per batch
NCORES = 8
CORES_PER_B = NCORES // B    # 4
RPC = M // CORES_PER_B       # 16384 rows per core

F32 = mybir.dt.float32
F32R = mybir.dt.float32r
EXP = mybir.ActivationFunctionType.Exp

# const layout per partition: [ ones(2) | ident(128) | wt(4*256) ]
# (ones is 2 wide: fp32r matmul dst patterns need 8-byte/2-elem granularity,
# so the denominator matmul computes 2 identical columns)
C_ONES, C_ID, C_WT = 0, 2, 130
C_TOT = 130 + 4 * K


def round_f32r(a: np.ndarray) -> np.ndarray:
    """fp32 -> fp32r rounding (RNE to 11 mantissa bits), bit-exact vs
    libwalrus fp32_to_fp32r."""
    b = np.ascontiguousarray(a, dtype=np.float32).view(np.uint32)
    r = (b + np.uint32(0x7FF) + ((b >> np.uint32(12)) & np.uint32(1))) & np.uint32(
        0xFFFFF000
    )
    return r.view(np.float32)


def build_nc(rows: int, subs_per_blk: int = 4) -> bass.Bass:
    """Emit the per-core bass program for `rows` m-rows."""
    assert rows % (128 * subs_per_blk) == 0
    nsub = rows // 128
    nblk = nsub // subs_per_blk

    nc = bacc.Bacc("TRN2", target_bir_lowering=False, debug=False)
    x_d = nc.dram_tensor("x", [rows, E], F32R, kind="ExternalInput")
    c_d = nc.dram_tensor("consts", [128, C_TOT], F32R, kind="ExternalInput")
    u_d = nc.dram_tensor("u", [2, 128, E], F32, kind="ExternalOutput")
    d_d = nc.dram_tensor("d", [128, 2, 2], F32, kind="ExternalOutput")

    with tile.TileContext(nc) as tc:
        with (
            tc.tile_pool(name="const", bufs=1) as constp,
            tc.tile_pool(name="xin", bufs=3) as xinp,
            tc.tile_pool(name="xt", bufs=2) as xtp,
            tc.tile_pool(name="wexp", bufs=2) as wexpp,
            tc.tile_pool(name="outs", bufs=1) as outp,
            tc.tile_pool(name="ps_t", bufs=2, space="PSUM") as ps_t,
            tc.tile_pool(name="ps_sc", bufs=2, space="PSUM") as ps_sc,
            tc.tile_pool(name="ps_acc", bufs=1, space="PSUM") as ps_acc,
        ):
            consts = constp.tile([128, C_TOT], F32R)
            nc.sync.dma_start(out=consts[:], in_=c_d.ap())
            ones = consts[:, C_ONES : C_ONES + 2]
            ident = consts[:, C_ID : C_ID + 128]

            u_ps = ps_acc.tile([128, 2, E], F32)    # 2 banks, live all kernel
            den_ps = ps_acc.tile([128, 2, 2], F32)  # 1 bank; [:, c, :] pairs

            for blk in range(nblk):
                xb = xinp.tile([128, subs_per_blk, E], F32R, tag="xb")
                r0 = blk * subs_per_blk * 128
                nc.sync.dma_start(
                    out=xb[:],
                    in_=x_d.ap()[r0 : r0 + subs_per_blk * 128, :].rearrange(
                        "(j p) e -> p j e", p=128
                    ),
                )
                for j in range(subs_per_blk):
                    it = blk * subs_per_blk + j
                    first, last = it == 0, it == nsub - 1
                    # -- transpose x subtile [128m, 512e] -> 4x [128e, 128m]
                    xt_ps = ps_t.tile([128, 4, 128], F32R, tag="xtps")
                    for ec in range(4):
                        nc.tensor.transpose(
                            xt_ps[:, ec, :],
                            xb[:, j, ec * 128 : (ec + 1) * 128],
                            ident,
                        )
                    xt_sb = xtp.tile([128, 4, 128], F32R, tag="xtsb")
                    nc.vector.tensor_copy(xt_sb[:], xt_ps[:])
                    # -- mm1: scores[m,k] = sum_e x[m,e] WT[e,k]
                    sc_ps = ps_sc.tile([128, K], F32, tag="scps")
                    for ec in range(4):
                        nc.tensor.matmul(
                            sc_ps[:],
                            xt_sb[:, ec, :],
                            consts[:, C_WT + ec * K : C_WT + (ec + 1) * K],
                            start=(ec == 0),
                            stop=(ec == 3),
                        )
                    # -- exp (tau=1, bias cancels)
                    wexp = wexpp.tile([128, K], F32R, tag="wexp")
                    nc.scalar.activation(wexp[:], sc_ps[:], EXP)
                    # -- mm2: U[k,e] += wexp^T @ x ; den[k] += wexp^T @ 1
                    for c in range(2):
                        wchunk = wexp[:, c * 128 : (c + 1) * 128]
                        nc.tensor.matmul(
                            u_ps[:, c, :],
                            wchunk,
                            xb[:, j, :],
                            start=first,
                            stop=last,
                        )
                        # start=True clears has_written for the WHOLE bank;
                        # both den chunks share one bank, so only the first
                        # chunk may issue it (chunk 1 then overwrites where
                        # bits are unset, which is the same as start).
                        nc.tensor.matmul(
                            den_ps[:, c, :],
                            wchunk,
                            ones,
                            start=first and c == 0,
                            stop=last,
                        )

            u_sb = outp.tile([128, 2, E], F32)
            den_sb = outp.tile([128, 2, 2], F32)
            nc.vector.tensor_copy(u_sb[:], u_ps[:])
            nc.vector.tensor_copy(den_sb[:], den_ps[:])
            nc.sync.dma_start(
                out=u_d.ap().rearrange("c p e -> p c e"), in_=u_sb[:]
            )
            nc.sync.dma_start(out=d_d.ap(), in_=den_sb[:])
    nc.compile()
    return nc


def _run(nc: bass.Bass, in_maps, **kw):
    return run_bass_kernel_spmd(nc, in_maps, list(range(len(in_maps))), **kw)


def make_consts(W: np.ndarray) -> np.ndarray:
    """[ones | identity | W.T as [c p] k chunks] per partition, f32r."""
    consts = np.zeros((128, C_TOT), np.float32)
    consts[:, C_ONES : C_ONES + 2] = 1.0
    consts[:, C_ID : C_ID + 128] = np.eye(128, dtype=np.float32)
    wt = round_f32r(np.ascontiguousarray(W.T, np.float32))  # [E, K]
    for c in range(4):
        consts[:, C_WT + c * K : C_WT + (c + 1) * K] = wt[
            c * 128 : (c + 1) * 128, :
        ]
    return consts


def make_in_maps(x: np.ndarray, W: np.ndarray):
    xf = round_f32r(np.ascontiguousarray(x, np.float32)).reshape(B, M, E)
    consts = make_consts(W)
    in_maps = []
    for i in range(NCORES):
        bi, si = divmod(i, CORES_PER_B)
        shard = np.ascontiguousarray(xf[bi, si * RPC : (si + 1) * RPC])
        in_maps.append({"x": shard, "consts": consts})
    return in_maps


def combine(results) -> np.ndarray:
    """Sum per-core partials per batch, normalize, stack."""
    out = np.empty((B, K, E), np.float32)
    for bi in range(B):
        U = np.zeros((K, E), np.float64)
        den = np.zeros((K,), np.float64)
        for si in range(CORES_PER_B):
            r = results[bi * CORES_PER_B + si]
            U += r["u"].reshape(K, E).astype(np.float64)
            # d is [128, 2, 2]: [p, c, dup] -> k = c*128 + p, drop dup col
            den += r["d"][:, :, 0].T.reshape(K).astype(np.float64)
        out[bi] = (U / den[:, None]).astype(np.float32)
    return out


_NC_CACHE: dict[int, bass.Bass] = {}


def kernel(x: np.ndarray, W: np.ndarray, b_bias: np.ndarray) -> np.ndarray:
    # b_bias shifts every column of scores by a constant along the softmax
    # axis -> cancels in softmax; unused by construction.
    if RPC not in _NC_CACHE:
        _NC_CACHE[RPC] = build_nc(RPC)
    res = _run(_NC_CACHE[RPC], make_in_maps(np.asarray(x), np.asarray(W)))
    return combine(res.results)
